# revision 1
# baseline (speedup 1.0000x reference)
"""Single-level 2D Haar DWT (pywt dwt2-compatible) on 8 TRN2 NeuronCores.

Input  x:   (32, 3, 512, 512) f32
Output out: (32, 12, 256, 256) f32, channel layout [LL, LH, HL, HH] per input
channel.

Sharding: pure data parallel — batch 32 -> 4 samples per core on 8 cores.

Per-core layout: the 12 images (4 samples x 3 channels) are viewed as a
(6144, 512) row matrix. A compute group is one sample (M=3 images, 3 MiB)
loaded in a single fully-contiguous DMA: partition p holds rows 4p..4p+3 of
each image, i.e. two 2x2-block row-pairs (k in {0,1}), both row parities
(t in {0,1}).

Compute per group (all row/column pairing done with strided SBUF views):
  ACT:  O' = 0.5 * R[odd rows]            (scalar engine, frees DVE)
  DVE:  s_e = E[::2] + E[1::2]            (column sum,  even rows, unscaled)
        d_e = E[::2] - E[1::2]
        s_o = O'[::2] + O'[1::2]          (already carry the 1/2)
        d_o = O'[::2] - O'[1::2]
  DVE:  LL = 0.5*s_e + s_o                (scalar_tensor_tensor folds the
        LH = 0.5*s_e - s_o                 remaining /2, no extra pass)
        HL = 0.5*d_e + d_o
        HH = 0.5*d_e - d_o
Output quadrant planes are staged so each image's 4 planes leave as one
1 MiB DMA with 2 KiB-contiguous per-partition chunks.
"""

import numpy as np

import concourse.bacc as bacc
import concourse.tile as tile
from concourse import mybir
from concourse.bass_utils import run_bass_kernel_spmd

N_CORES = 8
B, C, H, W = 32, 3, 512, 512
BPC = B // N_CORES          # samples per core
IMGS = BPC * C              # images per core
M = C                       # images per compute group (one sample)
G = IMGS // M               # groups per core
ROWS = IMGS * H             # 6144 input rows per core
HALF_W = W // 2
OUT_ROWS = IMGS * 4 * (H // 2)  # 12288 output rows per core

_FP32 = mybir.dt.float32
_ALU = mybir.AluOpType


def build(repeat: int = 1):
    """Build and compile the per-core Bass program. repeat>1 re-runs the whole
    body back to back (used for on-hardware timing)."""
    nc = bacc.Bacc("TRN2", debug=False, num_devices=N_CORES)
    x = nc.dram_tensor("x", [ROWS, W], _FP32, kind="ExternalInput")
    out = nc.dram_tensor("out", [OUT_ROWS, HALF_W], _FP32, kind="ExternalOutput")

    # input row  = ((g*M + m)*128 + p)*4 + r,  r = 2k + t (k row-pair, t parity)
    xv = x.ap().rearrange("(g m p r) w -> g p m r w", g=G, m=M, p=128, r=4)
    # output row = (((g*M + m)*4 + q)*128 + p)*2 + k   (q = quadrant LL/LH/HL/HH)
    ov = out.ap().rearrange(
        "(g m q p k) j -> g m p q k j", g=G, m=M, q=4, p=128, k=2
    )

    with tile.TileContext(nc) as tc:
        with (
            tc.tile_pool(name="io", bufs=2) as io_pool,
            tc.tile_pool(name="mid", bufs=2) as mid_pool,
        ):
            for _ in range(repeat):
                for g in range(G):
                    R = io_pool.tile([128, M * 4 * W], _FP32, tag="R")
                    nc.sync.dma_start(
                        out=R.rearrange("p (m r w) -> p m r w", m=M, r=4),
                        in_=xv[g],
                    )
                    # [p, m, k, t, j, u]: k row-pair, t row parity, u col parity
                    Rv = R.rearrange(
                        "p (m k t j u) -> p m k t j u", m=M, k=2, t=2, j=HALF_W, u=2
                    )

                    # 0.5 * odd rows -> O2 [p, m, k, w]
                    O2 = mid_pool.tile([128, M * 2 * W], _FP32, tag="O2")
                    O2w = O2.rearrange("p (m k w) -> p m k w", m=M, k=2)
                    nc.scalar.mul(
                        O2w,
                        R.rearrange("p (m k t w) -> p m k t w", m=M, k=2, t=2)[
                            :, :, :, 1
                        ],
                        0.5,
                    )
                    O2v = O2.rearrange(
                        "p (m k j u) -> p m k j u", m=M, k=2, j=HALF_W, u=2
                    )

                    se = mid_pool.tile([128, M * 2 * HALF_W], _FP32, tag="se")
                    de = mid_pool.tile([128, M * 2 * HALF_W], _FP32, tag="de")
                    so = mid_pool.tile([128, M * 2 * HALF_W], _FP32, tag="so")
                    do = mid_pool.tile([128, M * 2 * HALF_W], _FP32, tag="do")
                    sev = se.rearrange("p (m k j) -> p m k j", m=M, k=2)
                    dev = de.rearrange("p (m k j) -> p m k j", m=M, k=2)
                    sov = so.rearrange("p (m k j) -> p m k j", m=M, k=2)
                    dov = do.rearrange("p (m k j) -> p m k j", m=M, k=2)

                    Ee = Rv[:, :, :, 0, :, 0]  # even row, even col
                    Eo = Rv[:, :, :, 0, :, 1]  # even row, odd col
                    nc.vector.tensor_add(sev, Ee, Eo)
                    nc.vector.tensor_sub(dev, Ee, Eo)
                    nc.vector.tensor_add(sov, O2v[:, :, :, :, 0], O2v[:, :, :, :, 1])
                    nc.vector.tensor_sub(dov, O2v[:, :, :, :, 0], O2v[:, :, :, :, 1])

                    Q = mid_pool.tile([128, M * 4 * 2 * HALF_W], _FP32, tag="Q")
                    Qv = Q.rearrange("p (m q k j) -> p m q k j", m=M, q=4, k=2)
                    for q, (a, b_, op1) in enumerate(
                        [
                            (sev, sov, _ALU.add),
                            (sev, sov, _ALU.subtract),
                            (dev, dov, _ALU.add),
                            (dev, dov, _ALU.subtract),
                        ]
                    ):
                        nc.vector.scalar_tensor_tensor(
                            Qv[:, :, q], a, 0.5, b_, _ALU.mult, op1
                        )

                    # Stores go out on the scalar engine's HWDGE ring so they
                    # don't serialize behind the loads on the sync ring.
                    for m in range(M):
                        nc.scalar.dma_start(out=ov[g, m], in_=Qv[:, m])

    nc.compile()
    return nc


_NC_CACHE: dict[int, object] = {}


def _get_nc(repeat: int = 1):
    if repeat not in _NC_CACHE:
        _NC_CACHE[repeat] = build(repeat)
    return _NC_CACHE[repeat]


def kernel(x: np.ndarray) -> np.ndarray:
    x = np.asarray(x, dtype=np.float32)
    assert x.shape == (B, C, H, W)
    nc = _get_nc()
    in_maps = [
        {"x": np.ascontiguousarray(x[c * BPC : (c + 1) * BPC]).reshape(ROWS, W)}
        for c in range(N_CORES)
    ]
    res = run_bass_kernel_spmd(nc, in_maps, list(range(N_CORES)))
    shards = [
        res.results[c]["out"].reshape(BPC, C * 4, H // 2, W // 2)
        for c in range(N_CORES)
    ]
    return np.concatenate(shards, axis=0)



# revision 4
# speedup vs baseline: 1.2779x; 1.2779x over previous
"""Single-level 2D Haar DWT (pywt dwt2-compatible) on 8 TRN2 NeuronCores.

Input  x:   (32, 3, 512, 512) f32
Output out: (32, 12, 256, 256) f32, channel layout [LL, LH, HL, HH] per input
channel.

Sharding: pure data parallel — batch 32 -> 4 samples per core on 8 cores.

Per-core layout: the 12 images (4 samples x 3 channels) are viewed as a
(6144, 512) row matrix. A compute group is A=2 images side by side in the
partition dim (partition P = a*64 + p8): partition P holds 8 consecutive
rows of image 2g+a, so the group load is one fully contiguous 2 MiB DMA
with a single 16 KiB descriptor per partition.

Compute per group (strided SBUF views do all row/column pairing):
  ACT:  X  = 0.5 * R                       (one pass; folds all scaling)
  DVE:  se = X[even rows, even cols] + X[even, odd]
        de = X[even, even] - X[even, odd]
        so = X[odd,  even] + X[odd,  odd]
        do = X[odd,  even] - X[odd,  odd]
  DVE:  LL = se + so   LH = se - so        (all full-rate tensor_tensor;
        HL = de + do   HH = de - do         no half-rate fp32 stt)
Each partition then holds 4 consecutive output rows of each quadrant plane,
so the group store is one 2 MiB DMA with 4 KiB-contiguous chunks.
"""

import numpy as np

import concourse.bacc as bacc
import concourse.tile as tile
from concourse import mybir
from concourse.bass_utils import run_bass_kernel_spmd

N_CORES = 8
B, C, H, W = 32, 3, 512, 512
BPC = B // N_CORES          # samples per core
IMGS = BPC * C              # images per core
A = 2                       # images per compute group (in partition dim)
G = IMGS // A               # groups per core
P8 = 128 // A               # partitions per image
RPP = H // P8               # input rows per partition (8)
KP = RPP // 2               # 2x2-block row pairs per partition (4)
ROWS = IMGS * H             # 6144 input rows per core
HALF_W = W // 2
OUT_ROWS = IMGS * 4 * (H // 2)  # 12288 output rows per core

_FP32 = mybir.dt.float32


def build(repeat: int = 1):
    """Build and compile the per-core Bass program. repeat>1 re-runs the whole
    body back to back (used for on-hardware timing)."""
    nc = bacc.Bacc("TRN2", debug=False, num_devices=N_CORES)
    x = nc.dram_tensor("x", [ROWS, W], _FP32, kind="ExternalInput")
    out = nc.dram_tensor("out", [OUT_ROWS, HALF_W], _FP32, kind="ExternalOutput")

    # input row = g*1024 + P*8 + r: each group is a contiguous 2 MiB block,
    # partition-major, 16 KiB contiguous per partition.
    xv = x.ap().rearrange("(g P r) w -> g P r w", g=G, P=128, r=RPP)
    # output row = ((n*4 + q)*P8 + p)*KP + k  (n = image, q = quadrant)
    ov = out.ap().rearrange(
        "(n q p k) j -> n p q k j", n=IMGS, q=4, p=P8, k=KP
    )

    with tile.TileContext(nc) as tc:
        with (
            tc.tile_pool(name="io", bufs=3) as io_pool,
            tc.tile_pool(name="mid", bufs=2) as mid_pool,
        ):
            for _ in range(repeat):
                for g in range(G):
                    R = io_pool.tile([128, RPP * W], _FP32, tag="R")
                    nc.sync.dma_start(
                        out=R.rearrange("p (r w) -> p r w", r=RPP), in_=xv[g]
                    )

                    X = mid_pool.tile([128, RPP * W], _FP32, tag="X")
                    nc.scalar.mul(X, R, 0.5)
                    # [p, k, t, j, u]: k row-pair, t row parity, u col parity
                    Xv = X.rearrange(
                        "p (k t j u) -> p k t j u", k=KP, t=2, j=HALF_W, u=2
                    )

                    se = mid_pool.tile([128, KP * HALF_W], _FP32, tag="se")
                    de = mid_pool.tile([128, KP * HALF_W], _FP32, tag="de")
                    so = mid_pool.tile([128, KP * HALF_W], _FP32, tag="so")
                    do = mid_pool.tile([128, KP * HALF_W], _FP32, tag="do")
                    sev = se.rearrange("p (k j) -> p k j", k=KP)
                    dev = de.rearrange("p (k j) -> p k j", k=KP)
                    sov = so.rearrange("p (k j) -> p k j", k=KP)
                    dov = do.rearrange("p (k j) -> p k j", k=KP)

                    Ee = Xv[:, :, 0, :, 0]
                    Eo = Xv[:, :, 0, :, 1]
                    Oe = Xv[:, :, 1, :, 0]
                    Oo = Xv[:, :, 1, :, 1]
                    nc.vector.tensor_add(sev, Ee, Eo)
                    nc.vector.tensor_sub(dev, Ee, Eo)
                    nc.vector.tensor_add(sov, Oe, Oo)
                    nc.vector.tensor_sub(dov, Oe, Oo)

                    Q = mid_pool.tile([128, 4 * KP * HALF_W], _FP32, tag="Q")
                    Qv = Q.rearrange("p (q k j) -> p q k j", q=4, k=KP)
                    nc.vector.tensor_add(Qv[:, 0], sev, sov)
                    nc.vector.tensor_sub(Qv[:, 1], sev, sov)
                    nc.vector.tensor_add(Qv[:, 2], dev, dov)
                    nc.vector.tensor_sub(Qv[:, 3], dev, dov)

                    # Stores go out on the scalar engine's HWDGE ring so they
                    # don't serialize behind the loads on the sync ring. One
                    # 1 MiB DMA per image (64 partitions, 4 KiB chunks).
                    for a in range(A):
                        nc.scalar.dma_start(
                            out=ov[g * A + a],
                            in_=Qv[a * P8 : (a + 1) * P8],
                        )

    nc.compile()
    return nc


_NC_CACHE: dict[int, object] = {}


def _get_nc(repeat: int = 1):
    if repeat not in _NC_CACHE:
        _NC_CACHE[repeat] = build(repeat)
    return _NC_CACHE[repeat]


def kernel(x: np.ndarray) -> np.ndarray:
    x = np.asarray(x, dtype=np.float32)
    assert x.shape == (B, C, H, W)
    nc = _get_nc()
    in_maps = [
        {"x": np.ascontiguousarray(x[c * BPC : (c + 1) * BPC]).reshape(ROWS, W)}
        for c in range(N_CORES)
    ]
    res = run_bass_kernel_spmd(nc, in_maps, list(range(N_CORES)))
    shards = [
        res.results[c]["out"].reshape(BPC, C * 4, H // 2, W // 2)
        for c in range(N_CORES)
    ]
    return np.concatenate(shards, axis=0)
